# revision 8
# baseline (speedup 1.0000x reference)
"""Trainium2 Bass kernel for nn_AttnLayer_60636348285537.

Computes o = einsum('nt,bcthw->bcn', f, video) / (W*H) with the gaussian
attention filters f derived from mu_t/sigma_t, returning [B, C*N].

Sharding: pure data parallel over batch — B=8 batches on 8 NeuronCores,
one batch per core. Each core reduces its [C=1024, T*W*H=6272] slab.

Per-core pipeline:
  - gpsimd (SWDGE) casting DMAs stream the f32 video into bf16 SBUF tiles;
    the DMA-engine hold is charged on the bf16 output bytes, halving the
    stream vs an f32 copy (bf16 keeps rel err ~5e-3 << 2e-2 tol).
  - DVE stage 1 per chunk: pairwise fold adds (bf16 tensor_tensor runs in
    the 2x DVE perf mode) 196 -> 98 -> 49 -> 24 -> 12, a 1x reduce of the
    12-wide groups, plus the odd column 48: vs[c,t] = sum_wh v[c,t,wh].
  - The Activation engine owns ct0 and ct1's first half via per-timestep
    activation+accum ops (and the last two 1-timestep taper slices),
    freeing DVE headroom so it tracks the stream.
  - DVE stage 2: prod[c,n,t] = vs[c,t]*fs[n,t] (f32), one reduce per ct.
  - Stream order: ct7 bulk first, ct0 (Act) quarters, ct1..ct5 halves,
    ct6 in 8t granules, then a ct7 taper (4t/2t/1t/1t) so little work
    serializes after the last byte lands.
  - Output: SWDGE scatter-add on its own queue, prepared mid-stream and
    triggered after the last reduce (skips HWDGE+DGE latency in the
    tail). The out region is zeroed by an early DMA since PJRT output
    buffers are not reliably zero-initialized.
"""

import os
import sys

for _p in ("/opt/trn_rl_repo", "/root/.axon_site/_ro/trn_rl_repo"):
    if os.path.isdir(_p):
        sys.path.insert(0, _p)
        break

import numpy as np

P = 128          # SBUF partitions
C = 1024         # channels
T = 32           # time
WH = 196         # W*H = 14*14
X = T * WH       # free elems per channel
N = 3            # gaussian filters
N_CT = C // P    # channel tiles per core
N_CORES = 8
OUT_W = 64       # scatter-add row width (256B alignment); first 24 used

_cache = {}


def _build_module(vid_bufs=11, out_mode="dma", act_halves=3,
                  act_tail=2, tail_ts=(4, 2, 1, 1), ct6_grans=4):
    """act_halves: number of 16t half-ct granules owned by the Act engine
    (ct0 counts as two). act_tail: trailing 1t taper slices owned by Act."""
    import concourse.bacc as bacc
    import concourse.mybir as mybir
    from concourse import tile

    f32 = mybir.dt.float32
    bf16 = mybir.dt.bfloat16
    i16 = mybir.dt.int16
    XL = mybir.AxisListType.X
    COPY = mybir.ActivationFunctionType.Copy

    nc = bacc.Bacc("TRN2", target_bir_lowering=False, debug=False,
                   num_devices=N_CORES, num_swdge_queues=2)
    vid = nc.dram_tensor("video", [C, X], f32, kind="ExternalInput").ap()
    fw = nc.dram_tensor("fw", [P, N * T], f32, kind="ExternalInput").ap()
    if out_mode == "scatter":
        sidx = nc.dram_tensor("sidx", [16, 8], i16, kind="ExternalInput").ap()
        out = nc.dram_tensor("out", [P, OUT_W], f32,
                             kind="ExternalOutput").ap()
    else:
        out = nc.dram_tensor("out", [P, N_CT * N], f32,
                             kind="ExternalOutput").ap()

    vid_ct = vid.rearrange("(ct p) x -> ct p x", p=P)
    tail_ct = N_CT - 1
    bulk_t = T - sum(tail_ts)
    n_act_tail = min(act_tail, sum(1 for nt in tail_ts if nt == 1))

    # (ct, t0, nt, owner) in stream order
    plan = [(tail_ct, 0, 16, "dve"), (tail_ct, 16, bulk_t - 16, "dve")]
    plan += [(0, 8 * qt, 8, "act") for qt in range(4)]
    plan += [(1, 0, 16, "act" if act_halves >= 3 else "dve"),
             (1, 16, 16, "dve")]
    ct6 = N_CT - 2
    for ct in range(2, ct6):
        plan += [(ct, 0, 16, "dve"), (ct, 16, 16, "dve")]
    g = T // ct6_grans
    plan += [(ct6, g * i, g, "dve") for i in range(ct6_grans)]
    t0 = bulk_t
    n_ones = 0
    for nt in tail_ts:
        ones_left = sum(1 for x in tail_ts if x == 1) - n_ones
        owner = "act" if (nt == 1 and ones_left <= n_act_tail) else "dve"
        if nt == 1:
            n_ones += 1
        plan.append((tail_ct, t0, nt, owner))
        t0 += nt

    with nc.allow_low_precision(reason="bf16 pipeline, rel tol 2e-2"):
        with tile.TileContext(nc) as tc:
            with (
                tc.tile_pool(name="vid", bufs=vid_bufs) as vid_pool,
                tc.tile_pool(name="fold", bufs=2) as fold_pool,
                tc.tile_pool(name="persist", bufs=1) as persist,
                tc.tile_pool(name="tmp", bufs=2) as tmp_pool,
            ):
                f_sb = persist.tile([P, N * T], f32, tag="f_sb")
                f_view = f_sb.rearrange("p (n t) -> p n t", n=N)
                vs_all = persist.tile([P, N_CT * T], f32, tag="vs_all")
                vs_view = vs_all.rearrange("p (ct t) -> p ct t", t=T)
                prod7 = persist.tile([P, N * T], f32, tag="prod7")
                p7_view = prod7.rearrange("p (n t) -> p n t", n=N)
                if out_mode == "scatter":
                    out_sb = persist.tile([P, OUT_W], f32, tag="out_sb")
                    nc.gpsimd.memset(out_sb[:], 0.0)
                    idx_sb = persist.tile([16, 8], i16, tag="idx_sb")
                else:
                    out_sb = persist.tile([P, N_CT * N], f32, tag="out_sb")
                out_view = out_sb[:, :N_CT * N].rearrange(
                    "p (ct n) -> p ct n", n=N)
                # Act-private tiles: sharing vs_all/scrap with DVE would
                # serialize DVE behind the slower Act engine via tile deps
                scrap = persist.tile([P, WH], f32, tag="scrap")
                vsa = persist.tile([P, 3 * 16], f32, tag="vsa")
                vst = persist.tile([P, 4], f32, tag="vst")

                def stage1_dve(vt, ct, t0, nt):
                    """fold chain + reduce: vs[ct, t0:t0+nt] (f32)."""
                    ne = nt * WH
                    vs_dst = vs_view[:, ct, t0:t0 + nt]
                    if nt == 1:
                        nc.vector.reduce_sum(
                            vs_dst, vt[:, :ne].unsqueeze(1), axis=XL)
                        return
                    v3 = vt[:, :ne].rearrange(
                        "p (t two w) -> p t two w", two=2, w=98)
                    h = fold_pool.tile([P, 16 * 98], bf16, tag="h")
                    hu = h[:, :nt * 98]
                    nc.vector.tensor_add(
                        hu.rearrange("p (t w) -> p t w", w=98),
                        v3[:, :, 0, :], v3[:, :, 1, :])
                    h3 = hu.rearrange("p (t two w) -> p t two w", two=2, w=49)
                    q = fold_pool.tile([P, 16 * 49], bf16, tag="q")
                    qu = q[:, :nt * 49]
                    q_view = qu.rearrange("p (t w) -> p t w", w=49)
                    nc.vector.tensor_add(q_view, h3[:, :, 0, :],
                                         h3[:, :, 1, :])
                    if nt >= 8:
                        r = fold_pool.tile([P, 16 * 24], bf16, tag="r")
                        r_view = r[:, :nt * 24].rearrange(
                            "p (t w) -> p t w", w=24)
                        nc.vector.tensor_add(
                            r_view, q_view[:, :, 0:24], q_view[:, :, 24:48])
                        s = fold_pool.tile([P, 16 * 12], bf16, tag="s")
                        su = s[:, :nt * 12]
                        nc.vector.tensor_add(
                            su.rearrange("p (t w) -> p t w", w=12),
                            r_view[:, :, 0:12], r_view[:, :, 12:24])
                        vs0 = tmp_pool.tile([P, 16], bf16, tag="vs0")
                        nc.vector.reduce_sum(
                            vs0[:, :nt],
                            su.rearrange("p (t w) -> p t w", w=12), axis=XL)
                        nc.vector.tensor_add(vs_dst, vs0[:, :nt],
                                             q_view[:, :, 48])
                    else:
                        nc.vector.reduce_sum(vs_dst, q_view, axis=XL)

                def stage1_act(vt, ct, t0, nt):
                    for t in range(t0, t0 + nt):
                        if ct == tail_ct:
                            dst = vst[:, t - (T - 4):t - (T - 4) + 1]
                        else:
                            j = ct * 32 + t
                            dst = vsa[:, j:j + 1]
                        nc.scalar.activation(
                            scrap[:], vt[:, (t - t0) * WH:(t - t0 + 1) * WH],
                            COPY, accum_out=dst)

                act_ranges = {}  # ct -> list of (t0, t1) owned by Act

                def vs_src(ct, a, b):
                    for (ra, rb) in act_ranges.get(ct, ()):  # Act-owned
                        if ra <= a and b <= rb:
                            if ct == tail_ct:
                                return vst[:, a - (T - 4):b - (T - 4)]
                            return vsa[:, ct * 32 + a:ct * 32 + b]
                    return vs_view[:, ct, a:b]

                def premult_final(ct, pv=None, tslice=None, final=True):
                    if pv is None:
                        prod = tmp_pool.tile([P, N * T], f32, tag="prod")
                        pv = prod.rearrange("p (n t) -> p n t", n=N)
                    sl = slice(0, T) if tslice is None else tslice
                    # split the mul at Act/DVE ownership boundaries
                    cuts = {sl.start, sl.stop}
                    for (ra, rb) in act_ranges.get(ct, ()):
                        if sl.start < ra < sl.stop:
                            cuts.add(ra)
                        if sl.start < rb < sl.stop:
                            cuts.add(rb)
                    cuts = sorted(cuts)
                    for a, b in zip(cuts, cuts[1:]):
                        nc.vector.tensor_mul(
                            pv[:, :, a:b],
                            vs_src(ct, a, b).unsqueeze(1).broadcast_to(
                                [P, N, b - a]),
                            f_view[:, :, a:b])
                    if final:
                        nc.vector.reduce_sum(out_view[:, ct, :], pv[:],
                                             axis=XL)

                for ct_, t0_, nt_, owner_ in plan:
                    if owner_ == "act":
                        act_ranges.setdefault(ct_, []).append(
                            (t0_, t0_ + nt_))

                first = True
                for ct, t0, nt, owner in plan:
                    ne = nt * WH
                    vt = vid_pool.tile([P, X], bf16, tag="vt")
                    nc.gpsimd.dma_start(
                        vt[:, :ne], vid_ct[ct][:, t0 * WH:t0 * WH + ne])
                    if first:
                        nc.sync.dma_start(f_sb[:], fw[:])
                        if out_mode == "scatter":
                            nc.sync.dma_start(idx_sb[:], sidx)
                            # PJRT outputs are not reliably zeroed; the
                            # scatter-add needs a clean base
                            nc.sync.dma_start(out, out_sb[:])
                        first = False

                    if owner == "act":
                        stage1_act(vt, ct, t0, nt)
                        continue
                    stage1_dve(vt, ct, t0, nt)

                    if ct == tail_ct:
                        if t0 + nt == bulk_t:
                            premult_final(ct, pv=p7_view,
                                          tslice=slice(0, bulk_t),
                                          final=False)
                    elif t0 + nt == T:
                        premult_final(ct)
                        if ct == 5:
                            # Act finished ct0 by now; late placement keeps
                            # the sem wait out of the stream-tracking path
                            premult_final(0)
                        if ct == ct6:
                            premult_final(1)
                            if out_mode == "scatter":
                                # descriptor prep on the idle end of Pool's
                                # gen queue, isolated on SWDGE queue 1
                                nc.gpsimd.dma_scatter_add(
                                    out.unsqueeze(1), out_sb[:].unsqueeze(1),
                                    idx_sb[:], P, P, OUT_W,
                                    prepare_only=True,
                                    sem=nc.alloc_semaphore("out_sdma"),
                                    queue_num=1)

                premult_final(tail_ct, pv=p7_view,
                              tslice=slice(bulk_t, T), final=False)
                nc.vector.reduce_sum(out_view[:, tail_ct, :], p7_view[:],
                                     axis=XL)

                if out_mode == "scatter":
                    nc.gpsimd.trigger_dma(count=None, queue_num=1)
                else:
                    nc.sync.dma_start(out, out_sb[:])
    nc.compile()
    return nc


BEST = dict(vid_bufs=11, out_mode="dma", act_halves=3, act_tail=2,
            tail_ts=(4, 2, 1, 1), ct6_grans=4)


def _get_module():
    if "nc" not in _cache:
        _cache["nc"] = _build_module(**BEST)
    return _cache["nc"]


def _filters_scaled(mu_t: np.ndarray, sigma_t: np.ndarray) -> np.ndarray:
    """f / (W*H) as [N, T] float32, matching the reference filter math."""
    mu = np.tanh(mu_t.astype(np.float64))
    sg = 1.0 / (1.0 + np.exp(-sigma_t.astype(np.float64)))
    sigma = np.exp(1.5 - 2.0 * sg)
    centers = (T - 1) * (mu + 1.0) / 2.0
    t = np.arange(T, dtype=np.float64)[None, :] - centers[:, None]
    f = np.exp(-(t**2) / (2.0 * sigma[:, None] ** 2 + 1e-16))
    f = f / (np.sum(f, axis=1, keepdims=True) + 1e-16)
    return (f / WH).astype(np.float32)


def kernel(video: np.ndarray, mu_t: np.ndarray, sigma_t: np.ndarray,
           meta: np.ndarray) -> np.ndarray:
    from concourse import bass_utils

    B = video.shape[0]
    assert B == N_CORES, f"kernel hardcodes one batch per core, got B={B}"
    fs = _filters_scaled(np.asarray(mu_t), np.asarray(sigma_t))
    fw = np.tile(fs.reshape(1, N * T), (P, 1))
    vid = np.ascontiguousarray(np.asarray(video), dtype=np.float32)
    vid = vid.reshape(B, C, X)

    nc = _get_module()
    in_maps = []
    for b in range(B):
        m = {"video": vid[b], "fw": fw}
        if BEST["out_mode"] == "scatter":
            sidx = np.zeros((16, 8), np.int16)
            for i in range(P):
                sidx[i % 16, i // 16] = i
            m["sidx"] = sidx
        in_maps.append(m)
    res = bass_utils.run_bass_kernel_spmd(nc, in_maps,
                                          core_ids=list(range(N_CORES)))
    # out[p, ct*3+n] holds channel c = ct*128 + p
    outs = []
    for b in range(B):
        a = np.asarray(res.results[b]["out"])[:, :N_CT * N]
        a = a.reshape(P, N_CT, N)
        outs.append(a.transpose(1, 0, 2).reshape(C * N))
    return np.stack(outs).astype(np.float32)


# revision 10
# speedup vs baseline: 1.0497x; 1.0497x over previous
"""Trainium2 Bass kernel for nn_AttnLayer_60636348285537.

Computes o = einsum('nt,bcthw->bcn', f, video) / (W*H) with the gaussian
attention filters f derived from mu_t/sigma_t, returning [B, C*N].

Sharding: pure data parallel over batch — B=8 batches on 8 NeuronCores,
one batch per core. Each core reduces its [C=1024, T*W*H=6272] slab.

Per-core pipeline:
  - gpsimd (SWDGE) casting DMAs stream the f32 video into bf16 SBUF tiles;
    the DMA-engine hold is charged on the bf16 output bytes, halving the
    stream vs an f32 copy (bf16 keeps rel err ~5e-3 << 2e-2 tol).
  - DVE stage 1 per chunk: pairwise fold adds (bf16 tensor_tensor runs in
    the 2x DVE perf mode) 196 -> 98 -> 49 -> 24 -> 12, a 1x reduce of the
    12-wide groups, plus the odd column 48: vs[c,t] = sum_wh v[c,t,wh].
  - The Activation engine owns ct0 and ct1's first half via per-timestep
    activation+accum ops (and the last two 1-timestep taper slices),
    freeing DVE headroom so it tracks the stream.
  - DVE stage 2: prod[c,n,t] = vs[c,t]*fs[n,t] (f32), one reduce per ct.
  - Stream order: ct7 bulk first, ct0 (Act) quarters, ct1..ct5 halves,
    ct6 in 8t granules, then a ct7 taper (4t/2t/1t/1t) so little work
    serializes after the last byte lands.
  - Output: SWDGE scatter-add on its own queue, prepared mid-stream and
    triggered after the last reduce (skips HWDGE+DGE latency in the
    tail). The out region is zeroed by an early DMA since PJRT output
    buffers are not reliably zero-initialized.
"""

import os
import sys

for _p in ("/opt/trn_rl_repo", "/root/.axon_site/_ro/trn_rl_repo"):
    if os.path.isdir(_p):
        sys.path.insert(0, _p)
        break

import numpy as np

P = 128          # SBUF partitions
C = 1024         # channels
T = 32           # time
WH = 196         # W*H = 14*14
X = T * WH       # free elems per channel
N = 3            # gaussian filters
N_CT = C // P    # channel tiles per core
N_CORES = 8
OUT_W = 64       # scatter-add row width (256B alignment); first 24 used

_cache = {}


def _build_module(vid_bufs=11, out_mode="dma", act_halves=3,
                  act_tail=2, tail_ts=(4, 2, 1, 1), ct6_grans=4):
    """act_halves: number of 16t half-ct granules owned by the Act engine
    (ct0 counts as two). act_tail: trailing 1t taper slices owned by Act."""
    import concourse.bacc as bacc
    import concourse.mybir as mybir
    from concourse import tile

    f32 = mybir.dt.float32
    bf16 = mybir.dt.bfloat16
    i16 = mybir.dt.int16
    XL = mybir.AxisListType.X
    COPY = mybir.ActivationFunctionType.Copy

    nc = bacc.Bacc("TRN2", target_bir_lowering=False, debug=False,
                   num_devices=N_CORES, num_swdge_queues=2)
    vid = nc.dram_tensor("video", [C, X], f32, kind="ExternalInput").ap()
    fw = nc.dram_tensor("fw", [P, N * T], f32, kind="ExternalInput").ap()
    if out_mode == "scatter":
        sidx = nc.dram_tensor("sidx", [16, 8], i16, kind="ExternalInput").ap()
        out = nc.dram_tensor("out", [P, OUT_W], f32,
                             kind="ExternalOutput").ap()
    else:
        out = nc.dram_tensor("out", [P, N_CT * N], f32,
                             kind="ExternalOutput").ap()

    vid_ct = vid.rearrange("(ct p) x -> ct p x", p=P)
    tail_ct = N_CT - 1
    bulk_t = T - sum(tail_ts)
    n_act_tail = min(act_tail, sum(1 for nt in tail_ts if nt == 1))

    # (ct, t0, nt, owner) in stream order. Act granules (8t) are
    # interleaved at ~Act's digestion rate so neither engine starves.
    ct6 = N_CT - 2
    acts = [(0, 8 * qt, 8, "act") for qt in range(4)]
    if act_halves >= 3:
        acts += [(1, 0, 8, "act"), (1, 8, 8, "act")]
    dve16 = ([(1, 16, 16, "dve")] if act_halves >= 3
             else [(1, 0, 16, "dve"), (1, 16, 16, "dve")])
    for ct in range(2, ct6):
        dve16 += [(ct, 0, 16, "dve"), (ct, 16, 16, "dve")]
    plan = [(tail_ct, 0, 16, "dve"), (tail_ct, 16, bulk_t - 16, "dve")]
    ai, di = 0, 0
    while ai < len(acts) or di < len(dve16):
        if ai < len(acts):
            plan.append(acts[ai])
            ai += 1
        if di < len(dve16):
            plan.append(dve16[di])
            di += 1
    g = T // ct6_grans
    plan += [(ct6, g * i, g, "dve") for i in range(ct6_grans)]
    t0 = bulk_t
    n_ones = 0
    for nt in tail_ts:
        ones_left = sum(1 for x in tail_ts if x == 1) - n_ones
        owner = "act" if (nt == 1 and ones_left <= n_act_tail) else "dve"
        if nt == 1:
            n_ones += 1
        plan.append((tail_ct, t0, nt, owner))
        t0 += nt

    with nc.allow_low_precision(reason="bf16 pipeline, rel tol 2e-2"):
        with tile.TileContext(nc) as tc:
            with (
                tc.tile_pool(name="vid", bufs=vid_bufs) as vid_pool,
                tc.tile_pool(name="fold", bufs=2) as fold_pool,
                tc.tile_pool(name="persist", bufs=1) as persist,
                tc.tile_pool(name="tmp", bufs=2) as tmp_pool,
            ):
                f_sb = persist.tile([P, N * T], f32, tag="f_sb")
                f_view = f_sb.rearrange("p (n t) -> p n t", n=N)
                vs_all = persist.tile([P, N_CT * T], f32, tag="vs_all")
                vs_view = vs_all.rearrange("p (ct t) -> p ct t", t=T)
                prod7 = persist.tile([P, N * T], f32, tag="prod7")
                p7_view = prod7.rearrange("p (n t) -> p n t", n=N)
                if out_mode == "scatter":
                    out_sb = persist.tile([P, OUT_W], f32, tag="out_sb")
                    nc.gpsimd.memset(out_sb[:], 0.0)
                    idx_sb = persist.tile([16, 8], i16, tag="idx_sb")
                else:
                    out_sb = persist.tile([P, N_CT * N], f32, tag="out_sb")
                out_view = out_sb[:, :N_CT * N].rearrange(
                    "p (ct n) -> p ct n", n=N)
                # Act-private tiles: sharing vs_all/scrap with DVE would
                # serialize DVE behind the slower Act engine via tile deps
                scrap = persist.tile([P, WH], f32, tag="scrap")
                vsa = persist.tile([P, 3 * 16], f32, tag="vsa")
                vst = persist.tile([P, 4], f32, tag="vst")

                def stage1_dve(vt, ct, t0, nt):
                    """fold chain + reduce: vs[ct, t0:t0+nt] (f32)."""
                    ne = nt * WH
                    vs_dst = vs_view[:, ct, t0:t0 + nt]
                    if nt == 1:
                        nc.vector.reduce_sum(
                            vs_dst, vt[:, :ne].unsqueeze(1), axis=XL)
                        return
                    v3 = vt[:, :ne].rearrange(
                        "p (t two w) -> p t two w", two=2, w=98)
                    h = fold_pool.tile([P, 16 * 98], bf16, tag="h")
                    hu = h[:, :nt * 98]
                    nc.vector.tensor_add(
                        hu.rearrange("p (t w) -> p t w", w=98),
                        v3[:, :, 0, :], v3[:, :, 1, :])
                    h3 = hu.rearrange("p (t two w) -> p t two w", two=2, w=49)
                    q = fold_pool.tile([P, 16 * 49], bf16, tag="q")
                    qu = q[:, :nt * 49]
                    q_view = qu.rearrange("p (t w) -> p t w", w=49)
                    nc.vector.tensor_add(q_view, h3[:, :, 0, :],
                                         h3[:, :, 1, :])
                    if nt >= 8:
                        r = fold_pool.tile([P, 16 * 24], bf16, tag="r")
                        r_view = r[:, :nt * 24].rearrange(
                            "p (t w) -> p t w", w=24)
                        nc.vector.tensor_add(
                            r_view, q_view[:, :, 0:24], q_view[:, :, 24:48])
                        s = fold_pool.tile([P, 16 * 12], bf16, tag="s")
                        su = s[:, :nt * 12]
                        nc.vector.tensor_add(
                            su.rearrange("p (t w) -> p t w", w=12),
                            r_view[:, :, 0:12], r_view[:, :, 12:24])
                        vs0 = tmp_pool.tile([P, 16], bf16, tag="vs0")
                        nc.vector.reduce_sum(
                            vs0[:, :nt],
                            su.rearrange("p (t w) -> p t w", w=12), axis=XL)
                        nc.vector.tensor_add(vs_dst, vs0[:, :nt],
                                             q_view[:, :, 48])
                    else:
                        nc.vector.reduce_sum(vs_dst, q_view, axis=XL)

                def stage1_act(vt, ct, t0, nt):
                    for t in range(t0, t0 + nt):
                        if ct == tail_ct:
                            dst = vst[:, t - (T - 4):t - (T - 4) + 1]
                        else:
                            j = ct * 32 + t
                            dst = vsa[:, j:j + 1]
                        nc.scalar.activation(
                            scrap[:], vt[:, (t - t0) * WH:(t - t0 + 1) * WH],
                            COPY, accum_out=dst)

                act_ranges = {}  # ct -> list of (t0, t1) owned by Act

                def vs_src(ct, a, b):
                    for (ra, rb) in act_ranges.get(ct, ()):  # Act-owned
                        if ra <= a and b <= rb:
                            if ct == tail_ct:
                                return vst[:, a - (T - 4):b - (T - 4)]
                            return vsa[:, ct * 32 + a:ct * 32 + b]
                    return vs_view[:, ct, a:b]

                def premult_final(ct, pv=None, tslice=None, final=True):
                    if pv is None:
                        prod = tmp_pool.tile([P, N * T], f32, tag="prod")
                        pv = prod.rearrange("p (n t) -> p n t", n=N)
                    sl = slice(0, T) if tslice is None else tslice
                    # split the mul at Act/DVE ownership boundaries
                    cuts = {sl.start, sl.stop}
                    for (ra, rb) in act_ranges.get(ct, ()):
                        if sl.start < ra < sl.stop:
                            cuts.add(ra)
                        if sl.start < rb < sl.stop:
                            cuts.add(rb)
                    cuts = sorted(cuts)
                    for a, b in zip(cuts, cuts[1:]):
                        nc.vector.tensor_mul(
                            pv[:, :, a:b],
                            vs_src(ct, a, b).unsqueeze(1).broadcast_to(
                                [P, N, b - a]),
                            f_view[:, :, a:b])
                    if final:
                        nc.vector.reduce_sum(out_view[:, ct, :], pv[:],
                                             axis=XL)

                for ct_, t0_, nt_, owner_ in plan:
                    if owner_ == "act":
                        rs = act_ranges.setdefault(ct_, [])
                        if rs and rs[-1][1] == t0_:
                            rs[-1] = (rs[-1][0], t0_ + nt_)
                        else:
                            rs.append((t0_, t0_ + nt_))

                done = {}
                first = True
                for ct, t0, nt, owner in plan:
                    ne = nt * WH
                    vt = vid_pool.tile([P, X], bf16, tag="vt")
                    nc.gpsimd.dma_start(
                        vt[:, :ne], vid_ct[ct][:, t0 * WH:t0 * WH + ne])
                    if first:
                        nc.sync.dma_start(f_sb[:], fw[:])
                        if out_mode == "scatter":
                            nc.sync.dma_start(idx_sb[:], sidx)
                            # PJRT outputs are not reliably zeroed; the
                            # scatter-add needs a clean base
                            nc.sync.dma_start(out, out_sb[:])
                        first = False

                    if owner == "act":
                        stage1_act(vt, ct, t0, nt)
                        continue
                    stage1_dve(vt, ct, t0, nt)

                    done[ct] = done.get(ct, 0) + nt
                    if ct == tail_ct:
                        if t0 + nt == bulk_t:
                            premult_final(ct, pv=p7_view,
                                          tslice=slice(0, bulk_t),
                                          final=False)
                    elif done[ct] == T:
                        premult_final(ct)
                        if ct == 5 and 0 in act_ranges:
                            # Act finished ct0 by now; late placement keeps
                            # the sem wait out of the stream-tracking path
                            premult_final(0)
                        if ct == ct6 and done.get(ct6, 0) == T \
                                and 1 in act_ranges:
                            premult_final(1)

                premult_final(tail_ct, pv=p7_view,
                              tslice=slice(bulk_t, T), final=False)
                nc.vector.reduce_sum(out_view[:, tail_ct, :], p7_view[:],
                                     axis=XL)

                if out_mode == "scatter":
                    nc.gpsimd.dma_scatter_add(
                        out.unsqueeze(1), out_sb[:].unsqueeze(1),
                        idx_sb[:], P, P, OUT_W, prepare_only=True,
                        sem=nc.alloc_semaphore("out_sdma"), queue_num=1)
                    nc.gpsimd.trigger_dma(count=None, queue_num=1)
                else:
                    nc.sync.dma_start(out, out_sb[:])
    nc.compile()
    return nc


BEST = dict(vid_bufs=11, out_mode="dma", act_halves=3, act_tail=2,
            tail_ts=(4, 2, 1, 1), ct6_grans=4)


def _get_module():
    if "nc" not in _cache:
        _cache["nc"] = _build_module(**BEST)
    return _cache["nc"]


def _filters_scaled(mu_t: np.ndarray, sigma_t: np.ndarray) -> np.ndarray:
    """f / (W*H) as [N, T] float32, matching the reference filter math."""
    mu = np.tanh(mu_t.astype(np.float64))
    sg = 1.0 / (1.0 + np.exp(-sigma_t.astype(np.float64)))
    sigma = np.exp(1.5 - 2.0 * sg)
    centers = (T - 1) * (mu + 1.0) / 2.0
    t = np.arange(T, dtype=np.float64)[None, :] - centers[:, None]
    f = np.exp(-(t**2) / (2.0 * sigma[:, None] ** 2 + 1e-16))
    f = f / (np.sum(f, axis=1, keepdims=True) + 1e-16)
    return (f / WH).astype(np.float32)


def kernel(video: np.ndarray, mu_t: np.ndarray, sigma_t: np.ndarray,
           meta: np.ndarray) -> np.ndarray:
    from concourse import bass_utils

    B = video.shape[0]
    assert B == N_CORES, f"kernel hardcodes one batch per core, got B={B}"
    fs = _filters_scaled(np.asarray(mu_t), np.asarray(sigma_t))
    fw = np.tile(fs.reshape(1, N * T), (P, 1))
    vid = np.ascontiguousarray(np.asarray(video), dtype=np.float32)
    vid = vid.reshape(B, C, X)

    nc = _get_module()
    in_maps = []
    for b in range(B):
        m = {"video": vid[b], "fw": fw}
        if BEST["out_mode"] == "scatter":
            sidx = np.zeros((16, 8), np.int16)
            for i in range(P):
                sidx[i % 16, i // 16] = i
            m["sidx"] = sidx
        in_maps.append(m)
    res = bass_utils.run_bass_kernel_spmd(nc, in_maps,
                                          core_ids=list(range(N_CORES)))
    # out[p, ct*3+n] holds channel c = ct*128 + p
    outs = []
    for b in range(B):
        a = np.asarray(res.results[b]["out"])[:, :N_CT * N]
        a = a.reshape(P, N_CT, N)
        outs.append(a.transpose(1, 0, 2).reshape(C * N))
    return np.stack(outs).astype(np.float32)


# revision 12
# speedup vs baseline: 1.0671x; 1.0166x over previous
"""Trainium2 Bass kernel for nn_AttnLayer_60636348285537.

Computes o = einsum('nt,bcthw->bcn', f, video) / (W*H) with the gaussian
attention filters f derived from mu_t/sigma_t, returning [B, C*N].

Sharding: pure data parallel over batch — B=8 batches on 8 NeuronCores,
one batch per core. Each core reduces its [C=1024, T*W*H=6272] slab.

Per-core pipeline:
  - gpsimd (SWDGE) casting DMAs stream the f32 video into bf16 SBUF tiles;
    the DMA-engine hold is charged on the bf16 output bytes, halving the
    stream vs an f32 copy (bf16 keeps rel err ~5e-3 << 2e-2 tol).
  - DVE stage 1 per chunk: pairwise fold adds (bf16 tensor_tensor runs in
    the 2x DVE perf mode) 196 -> 98 -> 49 -> 24 -> 12, a 1x reduce of the
    12-wide groups, plus the odd column 48: vs[c,t] = sum_wh v[c,t,wh].
  - The Activation engine owns ct0 and ct1's first half via per-timestep
    activation+accum ops (and the last two 1-timestep taper slices),
    freeing DVE headroom so it tracks the stream.
  - DVE stage 2: prod[c,n,t] = vs[c,t]*fs[n,t] (f32), one reduce per ct.
  - Stream order: ct7 bulk first, ct0 (Act) quarters, ct1..ct5 halves,
    ct6 in 8t granules, then a ct7 taper (4t/2t/1t/1t) so little work
    serializes after the last byte lands.
  - Output: SWDGE scatter-add on its own queue, prepared mid-stream and
    triggered after the last reduce (skips HWDGE+DGE latency in the
    tail). The out region is zeroed by an early DMA since PJRT output
    buffers are not reliably zero-initialized.
"""

import os
import sys

for _p in ("/opt/trn_rl_repo", "/root/.axon_site/_ro/trn_rl_repo"):
    if os.path.isdir(_p):
        sys.path.insert(0, _p)
        break

import numpy as np

P = 128          # SBUF partitions
C = 1024         # channels
T = 32           # time
WH = 196         # W*H = 14*14
X = T * WH       # free elems per channel
N = 3            # gaussian filters
N_CT = C // P    # channel tiles per core
N_CORES = 8
OUT_W = 64       # scatter-add row width (256B alignment); first 24 used

_cache = {}


def _build_module(vid_bufs=11, out_mode="dma", act_halves=3,
                  act_tail=2, tail_ts=(8, 4, 2, 1, 1), ct6_grans=4):
    """act_halves: number of 16t half-ct granules owned by the Act engine
    (ct0 counts as two). act_tail: trailing 1t taper slices owned by Act."""
    import concourse.bacc as bacc
    import concourse.mybir as mybir
    from concourse import tile

    f32 = mybir.dt.float32
    bf16 = mybir.dt.bfloat16
    i16 = mybir.dt.int16
    XL = mybir.AxisListType.X
    COPY = mybir.ActivationFunctionType.Copy

    nc = bacc.Bacc("TRN2", target_bir_lowering=False, debug=False,
                   num_devices=N_CORES, num_swdge_queues=2)
    vid = nc.dram_tensor("video", [C, X], f32, kind="ExternalInput").ap()
    fw = nc.dram_tensor("fw", [P, N * T], f32, kind="ExternalInput").ap()
    if out_mode == "scatter":
        sidx = nc.dram_tensor("sidx", [16, 8], i16, kind="ExternalInput").ap()
        out = nc.dram_tensor("out", [P, OUT_W], f32,
                             kind="ExternalOutput").ap()
    else:
        out = nc.dram_tensor("out", [P, N_CT * N], f32,
                             kind="ExternalOutput").ap()

    vid_ct = vid.rearrange("(ct p) x -> ct p x", p=P)
    tail_ct = N_CT - 1
    bulk_t = T - sum(tail_ts)
    assert bulk_t == 16, tail_ts
    n_act_tail = min(act_tail, sum(1 for nt in tail_ts if nt == 1))

    # (ct, t0, nt, owner) granule list in stream order. Every 16t DVE
    # granule is paired with an Act granule (8t while Act is chain-bound
    # early, 4t once it's arrival-bound) so DVE's work rate stays below
    # the stream rate the whole way; ct6 streams as 8t granules and ct7
    # tapers 8/4/2/1/1 so almost nothing serializes after the last byte.
    ct6 = N_CT - 2
    acts = [(0, 0, 8, "act"), (0, 8, 8, "act"), (0, 16, 8, "act"),
            (0, 24, 4, "act"), (0, 28, 4, "act")]
    if act_halves >= 3:
        acts += [(1, 4 * i, 4, "act") for i in range(4)]
    dve16 = ([(1, 16, 16, "dve")] if act_halves >= 3
             else [(1, 0, 16, "dve"), (1, 16, 16, "dve")])
    for ct in range(2, ct6):
        dve16 += [(ct, 0, 16, "dve"), (ct, 16, 16, "dve")]
    plan = [(tail_ct, 0, 16, "dve")]
    ai = 0
    for d in dve16:
        plan.append(d)
        if ai < len(acts):
            plan.append(acts[ai])
            ai += 1
    plan += acts[ai:]
    g = T // ct6_grans
    plan += [(ct6, g * i, g, "dve") for i in range(ct6_grans)]
    t0 = 16
    n_ones = 0
    for nt in tail_ts:
        ones_left = sum(1 for x in tail_ts if x == 1) - n_ones
        owner = "act" if (nt == 1 and ones_left <= n_act_tail) else "dve"
        if nt == 1:
            n_ones += 1
        plan.append((tail_ct, t0, nt, owner))
        t0 += nt

    with nc.allow_low_precision(reason="bf16 pipeline, rel tol 2e-2"):
        with tile.TileContext(nc) as tc:
            with (
                tc.tile_pool(name="vid", bufs=vid_bufs) as vid_pool,
                tc.tile_pool(name="fold", bufs=2) as fold_pool,
                tc.tile_pool(name="persist", bufs=1) as persist,
                tc.tile_pool(name="tmp", bufs=2) as tmp_pool,
            ):
                f_sb = persist.tile([P, N * T], f32, tag="f_sb")
                f_view = f_sb.rearrange("p (n t) -> p n t", n=N)
                vs_all = persist.tile([P, N_CT * T], f32, tag="vs_all")
                vs_view = vs_all.rearrange("p (ct t) -> p ct t", t=T)
                prod7 = persist.tile([P, N * T], f32, tag="prod7")
                p7_view = prod7.rearrange("p (n t) -> p n t", n=N)
                if out_mode == "scatter":
                    out_sb = persist.tile([P, OUT_W], f32, tag="out_sb")
                    nc.gpsimd.memset(out_sb[:], 0.0)
                    idx_sb = persist.tile([16, 8], i16, tag="idx_sb")
                else:
                    out_sb = persist.tile([P, N_CT * N], f32, tag="out_sb")
                out_view = out_sb[:, :N_CT * N].rearrange(
                    "p (ct n) -> p ct n", n=N)
                # Act-private tiles: sharing vs_all/scrap with DVE would
                # serialize DVE behind the slower Act engine via tile deps
                scrap = persist.tile([P, WH], f32, tag="scrap")
                vsa = persist.tile([P, 3 * 16], f32, tag="vsa")
                vst = persist.tile([P, 4], f32, tag="vst")

                def stage1_dve(vt, ct, t0, nt):
                    """fold chain + reduce: vs[ct, t0:t0+nt] (f32)."""
                    ne = nt * WH
                    vs_dst = vs_view[:, ct, t0:t0 + nt]
                    if nt == 1:
                        nc.vector.reduce_sum(
                            vs_dst, vt[:, :ne].unsqueeze(1), axis=XL)
                        return
                    v3 = vt[:, :ne].rearrange(
                        "p (t two w) -> p t two w", two=2, w=98)
                    h = fold_pool.tile([P, 16 * 98], bf16, tag="h")
                    hu = h[:, :nt * 98]
                    nc.vector.tensor_add(
                        hu.rearrange("p (t w) -> p t w", w=98),
                        v3[:, :, 0, :], v3[:, :, 1, :])
                    h3 = hu.rearrange("p (t two w) -> p t two w", two=2, w=49)
                    q = fold_pool.tile([P, 16 * 49], bf16, tag="q")
                    qu = q[:, :nt * 49]
                    q_view = qu.rearrange("p (t w) -> p t w", w=49)
                    nc.vector.tensor_add(q_view, h3[:, :, 0, :],
                                         h3[:, :, 1, :])
                    if nt >= 8:
                        r = fold_pool.tile([P, 16 * 24], bf16, tag="r")
                        r_view = r[:, :nt * 24].rearrange(
                            "p (t w) -> p t w", w=24)
                        nc.vector.tensor_add(
                            r_view, q_view[:, :, 0:24], q_view[:, :, 24:48])
                        s = fold_pool.tile([P, 16 * 12], bf16, tag="s")
                        su = s[:, :nt * 12]
                        nc.vector.tensor_add(
                            su.rearrange("p (t w) -> p t w", w=12),
                            r_view[:, :, 0:12], r_view[:, :, 12:24])
                        vs0 = tmp_pool.tile([P, 16], bf16, tag="vs0")
                        nc.vector.reduce_sum(
                            vs0[:, :nt],
                            su.rearrange("p (t w) -> p t w", w=12), axis=XL)
                        nc.vector.tensor_add(vs_dst, vs0[:, :nt],
                                             q_view[:, :, 48])
                    else:
                        nc.vector.reduce_sum(vs_dst, q_view, axis=XL)

                def stage1_act(vt, ct, t0, nt):
                    for t in range(t0, t0 + nt):
                        if ct == tail_ct:
                            dst = vst[:, t - (T - 4):t - (T - 4) + 1]
                        else:
                            j = ct * 32 + t
                            dst = vsa[:, j:j + 1]
                        nc.scalar.activation(
                            scrap[:], vt[:, (t - t0) * WH:(t - t0 + 1) * WH],
                            COPY, accum_out=dst)

                act_ranges = {}  # ct -> list of (t0, t1) owned by Act

                def vs_src(ct, a, b):
                    for (ra, rb) in act_ranges.get(ct, ()):  # Act-owned
                        if ra <= a and b <= rb:
                            if ct == tail_ct:
                                return vst[:, a - (T - 4):b - (T - 4)]
                            return vsa[:, ct * 32 + a:ct * 32 + b]
                    return vs_view[:, ct, a:b]

                def premult_final(ct, pv=None, tslice=None, final=True):
                    if pv is None:
                        prod = tmp_pool.tile([P, N * T], f32, tag="prod")
                        pv = prod.rearrange("p (n t) -> p n t", n=N)
                    sl = slice(0, T) if tslice is None else tslice
                    # split the mul at Act/DVE ownership boundaries
                    cuts = {sl.start, sl.stop}
                    for (ra, rb) in act_ranges.get(ct, ()):
                        if sl.start < ra < sl.stop:
                            cuts.add(ra)
                        if sl.start < rb < sl.stop:
                            cuts.add(rb)
                    cuts = sorted(cuts)
                    for a, b in zip(cuts, cuts[1:]):
                        nc.vector.tensor_mul(
                            pv[:, :, a:b],
                            vs_src(ct, a, b).unsqueeze(1).broadcast_to(
                                [P, N, b - a]),
                            f_view[:, :, a:b])
                    if final:
                        nc.vector.reduce_sum(out_view[:, ct, :], pv[:],
                                             axis=XL)

                for ct_, t0_, nt_, owner_ in plan:
                    if owner_ == "act":
                        rs = act_ranges.setdefault(ct_, [])
                        if rs and rs[-1][1] == t0_:
                            rs[-1] = (rs[-1][0], t0_ + nt_)
                        else:
                            rs.append((t0_, t0_ + nt_))

                done = {}
                pm_done = {}
                first = True
                for ct, t0, nt, owner in plan:
                    ne = nt * WH
                    vt = vid_pool.tile([P, X], bf16, tag="vt")
                    nc.gpsimd.dma_start(
                        vt[:, :ne], vid_ct[ct][:, t0 * WH:t0 * WH + ne])
                    if first:
                        nc.sync.dma_start(f_sb[:], fw[:])
                        if out_mode == "scatter":
                            nc.sync.dma_start(idx_sb[:], sidx)
                            # PJRT outputs are not reliably zeroed; the
                            # scatter-add needs a clean base
                            nc.sync.dma_start(out, out_sb[:])
                        first = False

                    if owner == "act":
                        stage1_act(vt, ct, t0, nt)
                        continue
                    stage1_dve(vt, ct, t0, nt)

                    done[ct] = done.get(ct, 0) + nt
                    if ct == tail_ct:
                        if t0 + nt == bulk_t:
                            premult_final(ct, pv=p7_view,
                                          tslice=slice(0, bulk_t),
                                          final=False)
                        elif t0 + nt == T - 2 and n_act_tail == 2:
                            # DVE part of the taper premult; Act's last two
                            # timesteps are multiplied separately
                            premult_final(ct, pv=p7_view,
                                          tslice=slice(bulk_t, T - 2),
                                          final=False)
                    elif done[ct] == T:
                        premult_final(ct)
                    # Act-owned cts: premult placed at fixed points late in
                    # DVE program order (the Act data is ready by then, so
                    # the sem wait doesn't stall stream-tracking DVE ops)
                    if (ct, t0 + nt) == (5, 16) and 0 in act_ranges:
                        premult_final(0)
                    if (ct, t0 + nt) == (ct6, g) and 1 in act_ranges:
                        premult_final(1)

                last = slice(bulk_t if n_act_tail != 2 else T - 2, T)
                premult_final(tail_ct, pv=p7_view, tslice=last, final=False)
                nc.vector.reduce_sum(out_view[:, tail_ct, :], p7_view[:],
                                     axis=XL)

                if out_mode == "scatter":
                    nc.gpsimd.dma_scatter_add(
                        out.unsqueeze(1), out_sb[:].unsqueeze(1),
                        idx_sb[:], P, P, OUT_W, prepare_only=True,
                        sem=nc.alloc_semaphore("out_sdma"), queue_num=1)
                    nc.gpsimd.trigger_dma(count=None, queue_num=1)
                else:
                    nc.sync.dma_start(out, out_sb[:])
    nc.compile()
    return nc


BEST = dict(vid_bufs=11, out_mode="dma", act_halves=3, act_tail=2,
            tail_ts=(8, 4, 2, 1, 1), ct6_grans=4)


def _get_module():
    if "nc" not in _cache:
        _cache["nc"] = _build_module(**BEST)
    return _cache["nc"]


def _filters_scaled(mu_t: np.ndarray, sigma_t: np.ndarray) -> np.ndarray:
    """f / (W*H) as [N, T] float32, matching the reference filter math."""
    mu = np.tanh(mu_t.astype(np.float64))
    sg = 1.0 / (1.0 + np.exp(-sigma_t.astype(np.float64)))
    sigma = np.exp(1.5 - 2.0 * sg)
    centers = (T - 1) * (mu + 1.0) / 2.0
    t = np.arange(T, dtype=np.float64)[None, :] - centers[:, None]
    f = np.exp(-(t**2) / (2.0 * sigma[:, None] ** 2 + 1e-16))
    f = f / (np.sum(f, axis=1, keepdims=True) + 1e-16)
    return (f / WH).astype(np.float32)


def kernel(video: np.ndarray, mu_t: np.ndarray, sigma_t: np.ndarray,
           meta: np.ndarray) -> np.ndarray:
    from concourse import bass_utils

    B = video.shape[0]
    assert B == N_CORES, f"kernel hardcodes one batch per core, got B={B}"
    fs = _filters_scaled(np.asarray(mu_t), np.asarray(sigma_t))
    fw = np.tile(fs.reshape(1, N * T), (P, 1))
    vid = np.ascontiguousarray(np.asarray(video), dtype=np.float32)
    vid = vid.reshape(B, C, X)

    nc = _get_module()
    in_maps = []
    for b in range(B):
        m = {"video": vid[b], "fw": fw}
        if BEST["out_mode"] == "scatter":
            sidx = np.zeros((16, 8), np.int16)
            for i in range(P):
                sidx[i % 16, i // 16] = i
            m["sidx"] = sidx
        in_maps.append(m)
    res = bass_utils.run_bass_kernel_spmd(nc, in_maps,
                                          core_ids=list(range(N_CORES)))
    # out[p, ct*3+n] holds channel c = ct*128 + p
    outs = []
    for b in range(B):
        a = np.asarray(res.results[b]["out"])[:, :N_CT * N]
        a = a.reshape(P, N_CT, N)
        outs.append(a.transpose(1, 0, 2).reshape(C * N))
    return np.stack(outs).astype(np.float32)
